# revision 61
# baseline (speedup 1.0000x reference)
"""Trainium2 Bass kernel for nn_MemoryGraph (gnn_message_passing).

Self-contained: takes FULL inputs, shards across 8 NeuronCores internally,
returns the FULL output [BS, T, C, D].

Strategy (two SPMD launches, host glue between them):
  Phase 1 (8-way N-sharded): per-neuron modulator MLP for 512 neurons x 8
    batches per core; fc1_w streamed bf16 (DMA-bound), per-neuron matmuls
    on the PE, gates/norms/eff_* assembly on DVE/ACT.
  Phase 2 (v3, 4-way neuron x 2-way batch shard): 8-update message-passing
    scan, 1024 neurons x 4 batches per core.
    - neighbor gather via GPSIMD dma_gather with 512B elements (full DMA
      bandwidth) into partition layout p=(nsub16, s8)
    - sim dot products on DVE in bf16 2x mode (mult + halving tree)
    - the sigmoid gating multiply AND the 8-slot branch sum both run on
      the PE: stationary = w*msg rows [128, 64d], moving = sigma * static
      block-diagonal mask [128, 16]; psum out is [64(d), 16(neuron)]
      (stationary loads are free in the cost model)
    - ACT tanh evacuates psum; group combine + h update run d-major;
      an XBAR dma transpose re-partitions pm to row-major for the next
      update's gather source
    - pm exchanged between the 4 cores of a batch group with per-2-block
      AllGathers into ping-pong DRAM buffers (no WAR with own gathers),
      software-pipelined emission (stages lagged 1-2 blocks) so in-order
      engine queues never stall
    - the last update computes only neurons 0..63 (the only observable
      ones) and skips the exchange entirely
"""

import numpy as np
import ml_dtypes
from contextlib import ExitStack

import concourse.bass as bass
import concourse.tile as tile
from concourse import mybir, bacc, library_config
from concourse.bass_utils import run_bass_kernel_spmd

F32 = mybir.dt.float32
BF16 = mybir.dt.bfloat16
I32 = mybir.dt.int32
I16 = mybir.dt.int16
F8 = mybir.dt.float8e4
F16 = mybir.dt.float16
AF = mybir.ActivationFunctionType
OP = mybir.AluOpType

BS, T, C, N, K, D, H = 8, 32, 64, 4096, 32, 64, 64
NB, BSZ, NG, BPG = 4, 8, 1, 4
NCORES = 8
NS = N // NCORES  # neurons per core in phase 1 (512)

bf16 = ml_dtypes.bfloat16

_prog_cache = {}


# --------------------------------------------------------------------------
# Phase 2: B-sharded scan
# --------------------------------------------------------------------------
def build_phase2(U, NBLK=32, SLAB=2):
    """One batch per core. NBLK 128-neuron blocks, SLAB blocks per slab."""
    assert NBLK % SLAB == 0
    nS = NBLK // SLAB
    Nn = NBLK * 128
    nc = bacc.Bacc("TRN2", target_bir_lowering=False, debug=False,
                   num_devices=NCORES)

    # pm rows duplicated to 256B (dma_gather needs elem_size % 256B == 0)
    pm_init = nc.dram_tensor("pm_init", [Nn, 2 * D], BF16,
                             kind="ExternalInput")
    w_hbm = nc.dram_tensor("w_hbm", [nS, 128, SLAB, K, D], BF16,
                           kind="ExternalInput")
    key_in = nc.dram_tensor("key_nb", [128, NBLK, D], BF16,
                            kind="ExternalInput")
    effp_in = nc.dram_tensor("effp_nb", [128, NBLK, D], F32,
                             kind="ExternalInput")
    dec_in = nc.dram_tensor("dec1m_nb", [128, NBLK], F32,
                            kind="ExternalInput")  # 1 - eff_decay
    h_in = nc.dram_tensor("h0_nb", [128, NBLK, D], F32, kind="ExternalInput")
    g_in = nc.dram_tensor("g_nb", [128, NBLK, NB, D], BF16,
                          kind="ExternalInput")
    cc_in = nc.dram_tensor("cc_u", [C, U, D], F32, kind="ExternalInput")
    NIDX = SLAB * K * 128  # idxs per slab-gather
    idx_in = nc.dram_tensor("idx", [128, nS, NIDX // 16], I16,
                            kind="ExternalInput")
    out_t = nc.dram_tensor("out_pm", [C, U, D], F32, kind="ExternalOutput")

    with tile.TileContext(nc) as tc, ExitStack() as ctx:
        res = ctx.enter_context(tc.tile_pool(name="res", bufs=1))
        dram = ctx.enter_context(tc.tile_pool(name="dram", bufs=1,
                                              space="DRAM"))
        gp = ctx.enter_context(tc.tile_pool(name="gath", bufs=3))
        wp = ctx.enter_context(tc.tile_pool(name="wsl", bufs=2))
        bigp = ctx.enter_context(tc.tile_pool(name="big", bufs=2))
        sp = ctx.enter_context(tc.tile_pool(name="small", bufs=2))

        key_sb = res.tile([128, NBLK, D], BF16)
        nc.sync.dma_start(out=key_sb[:], in_=key_in.ap())
        effp_sb = res.tile([128, NBLK, D], F32)
        nc.sync.dma_start(out=effp_sb[:], in_=effp_in.ap())
        dec_sb = res.tile([128, NBLK], F32)
        nc.sync.dma_start(out=dec_sb[:], in_=dec_in.ap())
        h_sb = res.tile([128, NBLK, D], F32)
        nc.sync.dma_start(out=h_sb[:], in_=h_in.ap())
        g_sb = res.tile([128, NBLK, NB, D], BF16)
        nc.sync.dma_start(out=g_sb[:], in_=g_in.ap())
        cc_sb = res.tile([C, U, D], F32)
        nc.sync.dma_start(out=cc_sb[:], in_=cc_in.ap())
        pm_sb = res.tile([128, NBLK, D], BF16)
        out_sb = res.tile([C, U, D], F32)
        pm_dram = dram.tile([Nn, 2 * D], BF16)
        nc.gpsimd.load_library(library_config.mlp)

        for u in range(U):
            src = pm_init.ap() if u == 0 else pm_dram[:, :]
            for s in range(nS):
                sl = slice(s * SLAB, (s + 1) * SLAB)
                wl = wp.tile([128, SLAB, K, D], BF16)
                nc.sync.dma_start(out=wl[:], in_=w_hbm.ap()[s])
                idx_sl = wp.tile([128, NIDX // 16], I16, tag="idx")
                nc.sync.dma_start(out=idx_sl[:], in_=idx_in.ap()[:, s])
                mg = gp.tile([128, SLAB, K, 2 * D], BF16)
                nc.gpsimd.dma_gather(
                    out_ap=mg[:].rearrange("p a k e -> p (a k) e"),
                    in_ap=src, idxs_ap=idx_sl[:],
                    num_idxs=NIDX, num_idxs_reg=NIDX, elem_size=2 * D,
                    single_packet=False)

                # --- sim = sum_d(msg * key) ---
                tmp = bigp.tile([128, SLAB, K, D], BF16)
                keyb = key_sb[:, sl, :].unsqueeze(2).to_broadcast(
                    (128, SLAB, K, D))
                nc.vector.tensor_mul(tmp[:], mg[:, :, :, 0:D], keyb)
                r1 = sp.tile([128, SLAB, K, 32], BF16)
                nc.vector.tensor_add(r1[:], tmp[:, :, :, 0:32],
                                     tmp[:, :, :, 32:64])
                r2 = sp.tile([128, SLAB, K, 16], BF16)
                nc.vector.tensor_add(r2[:], r1[:, :, :, 0:16],
                                     r1[:, :, :, 16:32])
                r3 = sp.tile([128, SLAB, K, 8], BF16)
                nc.vector.tensor_add(r3[:], r2[:, :, :, 0:8],
                                     r2[:, :, :, 8:16])
                r4 = sp.tile([128, SLAB, K, 4], F32)
                nc.vector.tensor_add(r4[:], r3[:, :, :, 0:4],
                                     r3[:, :, :, 4:8])
                r5 = sp.tile([128, SLAB, K, 2], F32)
                nc.vector.tensor_add(r5[:], r4[:, :, :, 0:2],
                                     r4[:, :, :, 2:4])
                sim = sp.tile([128, SLAB, K, 1], F32)
                nc.vector.tensor_add(sim[:], r5[:, :, :, 0:1],
                                     r5[:, :, :, 1:2])

                # sigma duplicated to adjacent pairs so the sigma-broadcast
                # multiply stays in DVE 2x mode (packed reads need innermost
                # step 1 over >=2 elements).
                sg = sp.tile([128, SLAB, K, 2], BF16)
                nc.scalar.activation(
                    sg[:], sim[:].to_broadcast((128, SLAB, K, 2)), AF.Sigmoid)

                # --- contrib = msg * W * sigma ---
                wm = bigp.tile([128, SLAB, K, D], BF16)
                nc.vector.tensor_mul(wm[:], wl[:], mg[:, :, :, 0:D])
                ct = bigp.tile([128, SLAB, K, D], BF16)
                nc.vector.tensor_mul(
                    ct[:].rearrange("p a k (q t) -> p a k q t", t=2),
                    wm[:].rearrange("p a k (q t) -> p a k q t", t=2),
                    sg[:].unsqueeze(3).to_broadcast((128, SLAB, K, D // 2, 2)))

                # --- branch tree: sum over s (8) then tanh ---
                ctr = ct[:].rearrange("p s (j b) d -> p s j b d", j=NB)
                b1 = sp.tile([128, SLAB, NB, 4, D], BF16)
                nc.vector.tensor_add(b1[:], ctr[:, :, :, 0:4, :],
                                     ctr[:, :, :, 4:8, :])
                b2 = sp.tile([128, SLAB, NB, 2, D], BF16)
                nc.vector.tensor_add(b2[:], b1[:, :, :, 0:2, :],
                                     b1[:, :, :, 2:4, :])
                br = sp.tile([128, SLAB, NB, D], F32)
                nc.vector.tensor_add(br[:], b2[:, :, :, 0, :],
                                     b2[:, :, :, 1, :])
                brt = sp.tile([128, SLAB, NB, D], BF16)
                nc.scalar.activation(brt[:], br[:], AF.Tanh)

                # --- group: sum over j (4) then tanh ---
                gb = sp.tile([128, SLAB, NB, D], BF16)
                nc.vector.tensor_mul(gb[:], brt[:], g_sb[:, sl, :, :])
                g1 = sp.tile([128, SLAB, 2, D], BF16)
                nc.vector.tensor_add(g1[:], gb[:, :, 0:2, :],
                                     gb[:, :, 2:4, :])
                rcv = sp.tile([128, SLAB, D], F32)
                nc.vector.tensor_add(rcv[:], g1[:, :, 0, :], g1[:, :, 1, :])
                rct = sp.tile([128, SLAB, D], F32)
                nc.scalar.activation(rct[:], rcv[:], AF.Tanh)
                if s == 0:
                    nc.vector.tensor_add(rct[0:C, 0, :], rct[0:C, 0, :],
                                         cc_sb[:, u, :])

                # --- h update: h' = h + (1-d)*(r-h); pm = tanh(h'*effp) ---
                dd = sp.tile([128, SLAB, D], F32)
                nc.vector.tensor_sub(dd[:], rct[:], h_sb[:, sl, :])
                d2 = sp.tile([128, SLAB, D], F32)
                for j in range(SLAB):
                    nbi = s * SLAB + j
                    nc.vector.tensor_scalar(
                        d2[:, j, :], dd[:, j, :],
                        dec_sb[:, nbi:nbi + 1], None, OP.mult)
                nc.vector.tensor_add(h_sb[:, sl, :], h_sb[:, sl, :], d2[:])
                pmt = sp.tile([128, SLAB, D], F32)
                nc.vector.tensor_mul(pmt[:], h_sb[:, sl, :],
                                     effp_sb[:, sl, :])
                nc.scalar.activation(pm_sb[:, sl, :], pmt[:], AF.Tanh)
                if s == 0:
                    nc.scalar.activation(out_sb[:, u, :], pmt[0:C, 0, :],
                                         AF.Tanh)
            pmv = pm_dram[:, :].rearrange("(nb p) e -> p nb e", p=128)
            nc.sync.dma_start(out=pmv[:, :, 0:D], in_=pm_sb[:])
            nc.sync.dma_start(out=pmv[:, :, D:2 * D], in_=pm_sb[:])
        nc.sync.dma_start(out=out_t.ap(), in_=out_sb[:])

    nc.compile()
    return nc


def prep_phase2_inputs(b, eff_key, eff_prim, eff_decay, h, prev_messages,
                       cc_signals, conn, w_kmaj, g_nb, update_ts,
                       NBLK=32, SLAB=2):
    """Per-core (batch b) input map for phase 2. eff_* are full [BS,N,*]."""
    nS = NBLK // SLAB
    U = len(update_ts)

    def nb_layout(x):  # [N, ...] -> [128, NBLK, ...]
        return np.ascontiguousarray(
            x.reshape((NBLK, 128) + x.shape[1:]).swapaxes(0, 1))

    return {
        "pm_init": np.ascontiguousarray(
            np.concatenate([prev_messages[b], prev_messages[b]], axis=-1)
        ).astype(bf16),
        "w_hbm": w_kmaj,
        "key_nb": nb_layout(eff_key[b]).astype(bf16),
        "effp_nb": nb_layout(eff_prim[b]).astype(np.float32),
        "dec1m_nb": nb_layout(1.0 - eff_decay[b]).astype(np.float32),
        "h0_nb": nb_layout(h[b]).astype(np.float32),
        "g_nb": g_nb,
        "cc_u": np.ascontiguousarray(
            cc_signals[b][update_ts].transpose(1, 0, 2)).astype(np.float32),
        "idx": prep_idx(conn, NBLK, SLAB),
    }


def prep_idx(conn, NBLK=32, SLAB=2):
    """dma_gather idx order: idx i -> partition i%128, chunk i//128.
    Want mg[p, nb, k] = pm[conn[(s*SLAB+nb)*128 + p, k]]:
    i = (nb*K + k)*128 + p. Wrapped [16, n/16] then replicated to 128."""
    nS = NBLK // SLAB
    K_ = conn.shape[1]
    nidx = SLAB * K_ * 128
    out = np.empty((128, nS, nidx // 16), np.int16)
    for s in range(nS):
        blk = conn[s * SLAB * 128:(s + 1) * SLAB * 128].reshape(
            SLAB, 128, K_)  # [nb, p, k]
        flat = np.ascontiguousarray(blk.transpose(0, 2, 1)).reshape(-1)
        wrap = flat.reshape(-1, 16).T  # [16, nidx/16]
        out[:, s, :] = np.tile(wrap, (8, 1))
    return np.ascontiguousarray(out)


def prep_phase2_consts(dendrite_branch_w, dendrite_group_w, NBLK=32, SLAB=2):
    nS = NBLK // SLAB
    w = dendrite_branch_w.reshape(NBLK * 128, K, D)
    w_kmaj = np.ascontiguousarray(
        w.reshape(nS, SLAB, 128, K, D).transpose(0, 2, 1, 3, 4)).astype(bf16)
    g = dendrite_group_w.reshape(NBLK * 128, BPG, D)
    g_nb = np.ascontiguousarray(
        g.reshape(NBLK, 128, BPG, D).swapaxes(0, 1)).astype(bf16)
    return w_kmaj, g_nb



# --------------------------------------------------------------------------
# Phase 2 (N-sharded variant): 512 neurons x all 8 batches per core,
# pm all-gathered across cores each update. Gather elements are 1KB
# ([n, 8b, 64d] bf16 rows), so descriptor cost is 4x lower than the
# B-sharded variant, and the dendrite weights fit in SBUF.
# --------------------------------------------------------------------------
def build_phase2_ns(U):
    NBL2 = 4          # 128-neuron blocks per core
    QJ = NB           # branch quarters per block
    nc = bacc.Bacc("TRN2", target_bir_lowering=False, debug=False,
                   num_devices=NCORES)

    pm_init = nc.dram_tensor("pm_init", [N, BS, D], BF16,
                             kind="ExternalInput")
    w_in = nc.dram_tensor("w_nb", [128, NBL2, K, D], BF16,
                          kind="ExternalInput")
    key_in = nc.dram_tensor("key_nb", [128, NBL2, BS, D], BF16,
                            kind="ExternalInput")
    effp_in = nc.dram_tensor("effp_nb", [128, NBL2, BS, D], F32,
                             kind="ExternalInput")
    dec_in = nc.dram_tensor("dec1m_nb", [128, NBL2, BS], F32,
                            kind="ExternalInput")
    h_in = nc.dram_tensor("h0_nb", [128, NBL2, BS, D], F32,
                          kind="ExternalInput")
    g_in = nc.dram_tensor("g_nb", [128, NBL2, NB, D], BF16,
                          kind="ExternalInput")
    cc_in = nc.dram_tensor("cc_u", [C, U, BS, D], BF16,
                           kind="ExternalInput")
    idx_in = nc.dram_tensor("idx", [128, NBL2 * QJ, BSZ * 128 // 16], I16,
                            kind="ExternalInput")
    out_t = nc.dram_tensor("out_pm", [C, U, BS, D], F32,
                           kind="ExternalOutput")
    # pm_full row order is (nb, core, p): global neuron n = 512*c + 128*nb + p
    # lives at row nb*1024 + c*128 + p. Per-block AllGathers then write
    # contiguous stripes and pipeline behind the per-block compute.
    pm_slices = [nc.dram_tensor(f"pm_slice{i}", [128, BS, D], BF16)
                 for i in range(4)]
    pm_full = nc.dram_tensor("pm_full", [4, NCORES * 128, BS, D], BF16)

    with tile.TileContext(nc) as tc, ExitStack() as ctx:
        res = ctx.enter_context(tc.tile_pool(name="res", bufs=1))
        gp = ctx.enter_context(tc.tile_pool(name="gath", bufs=3))
        bigp = ctx.enter_context(tc.tile_pool(name="big", bufs=2))
        sp = ctx.enter_context(tc.tile_pool(name="small", bufs=2))

        nc.gpsimd.load_library(library_config.mlp)
        w_sb = res.tile([128, NBL2, K, D], BF16)
        nc.sync.dma_start(out=w_sb[:], in_=w_in.ap())
        key_sb = res.tile([128, NBL2, BS, D], BF16)
        nc.sync.dma_start(out=key_sb[:], in_=key_in.ap())
        effp_sb = res.tile([128, NBL2, BS, D], F32)
        nc.sync.dma_start(out=effp_sb[:], in_=effp_in.ap())
        dec_sb = res.tile([128, NBL2, BS], F32)
        nc.sync.dma_start(out=dec_sb[:], in_=dec_in.ap())
        h_sb = res.tile([128, NBL2, BS, D], F32)
        nc.sync.dma_start(out=h_sb[:], in_=h_in.ap())
        g_sb = res.tile([128, NBL2, NB, D], BF16)
        nc.sync.dma_start(out=g_sb[:], in_=g_in.ap())
        cc_sb = res.tile([C, U, BS, D], BF16)
        nc.sync.dma_start(out=cc_sb[:], in_=cc_in.ap())
        idx_sb = res.tile([128, NBL2 * QJ, BSZ * 128 // 16], I16)
        nc.sync.dma_start(out=idx_sb[:], in_=idx_in.ap())
        pm_sb = res.tile([128, NBL2, BS, D], BF16)

        NIDX = BSZ * 128  # idxs per gather (1024)
        for u in range(U):
            src = (pm_init.ap() if u == 0 else
                   pm_full.ap().rearrange("a c b d -> (a c) b d"))
            for nb in range(NBL2):
                brb = sp.tile([128, NB, BS, D], BF16, tag="brb")
                for j in range(QJ):
                    mg = gp.tile([128, BSZ, BS, D], BF16)
                    nc.gpsimd.dma_gather(
                        out_ap=mg[:].rearrange("p k b d -> p k (b d)"),
                        in_ap=src.rearrange("n b d -> n (b d)"),
                        idxs_ap=idx_sb[:, nb * QJ + j, :],
                        num_idxs=NIDX, num_idxs_reg=NIDX,
                        elem_size=BS * D)

                    ks = slice(j * BSZ, (j + 1) * BSZ)
                    # sim
                    tmp = bigp.tile([128, BSZ, BS, D], BF16)
                    keyb = key_sb[:, nb, :, :].unsqueeze(1).to_broadcast(
                        (128, BSZ, BS, D))
                    nc.vector.tensor_mul(tmp[:], mg[:], keyb)
                    r1 = sp.tile([128, BSZ, BS, 32], BF16)
                    nc.vector.tensor_add(r1[:], tmp[:, :, :, 0:32],
                                         tmp[:, :, :, 32:64])
                    r2 = sp.tile([128, BSZ, BS, 16], BF16)
                    nc.vector.tensor_add(r2[:], r1[:, :, :, 0:16],
                                         r1[:, :, :, 16:32])
                    r3 = sp.tile([128, BSZ, BS, 8], BF16)
                    nc.vector.tensor_add(r3[:], r2[:, :, :, 0:8],
                                         r2[:, :, :, 8:16])
                    r4 = sp.tile([128, BSZ, BS, 4], F32)
                    nc.vector.tensor_add(r4[:], r3[:, :, :, 0:4],
                                         r3[:, :, :, 4:8])
                    r5 = sp.tile([128, BSZ, BS, 2], F32)
                    nc.vector.tensor_add(r5[:], r4[:, :, :, 0:2],
                                         r4[:, :, :, 2:4])
                    sim = sp.tile([128, BSZ, BS, 1], F32)
                    nc.vector.tensor_add(sim[:], r5[:, :, :, 0:1],
                                         r5[:, :, :, 1:2])
                    sg = sp.tile([128, BSZ, BS, 2], BF16)
                    nc.scalar.activation(
                        sg[:], sim[:].to_broadcast((128, BSZ, BS, 2)),
                        AF.Sigmoid)

                    # contrib = msg * W * sigma  (W broadcast over b, on Pool)
                    wm = bigp.tile([128, BSZ, BS, D], BF16)
                    wb = w_sb[:, nb, ks, :].unsqueeze(2).to_broadcast(
                        (128, BSZ, BS, D))
                    nc.vector.tensor_mul(wm[:], mg[:], wb)
                    ct = bigp.tile([128, BSZ, BS, D], BF16, tag="tmp")
                    nc.vector.tensor_mul(
                        ct[:].rearrange("p k b (q t) -> p k b q t", t=2),
                        wm[:].rearrange("p k b (q t) -> p k b q t", t=2),
                        sg[:].unsqueeze(3).to_broadcast(
                            (128, BSZ, BS, D // 2, 2)))

                    # branch tree over k (8 -> 1), tanh
                    b1 = sp.tile([128, 4, BS, D], BF16)
                    nc.vector.tensor_add(b1[:], ct[:, 0:4, :, :],
                                         ct[:, 4:8, :, :])
                    b2 = sp.tile([128, 2, BS, D], BF16)
                    nc.vector.tensor_add(b2[:], b1[:, 0:2, :, :],
                                         b1[:, 2:4, :, :])
                    br = sp.tile([128, BS, D], F32)
                    nc.vector.tensor_add(br[:], b2[:, 0, :, :],
                                         b2[:, 1, :, :])
                    nc.scalar.activation(brb[:, j, :, :], br[:], AF.Tanh)

                # group combine for block nb
                gb = sp.tile([128, NB, BS, D], BF16, tag="b1")
                nc.vector.tensor_mul(
                    gb[:], brb[:],
                    g_sb[:, nb, :, :].unsqueeze(2).to_broadcast(
                        (128, NB, BS, D)))
                g1 = sp.tile([128, 2, BS, D], BF16)
                nc.vector.tensor_add(g1[:], gb[:, 0:2, :, :],
                                     gb[:, 2:4, :, :])
                rcv = sp.tile([128, BS, D], F32)
                nc.vector.tensor_add(rcv[:], g1[:, 0, :, :], g1[:, 1, :, :])
                rct = sp.tile([128, BS, D], F32)
                nc.scalar.activation(rct[:], rcv[:], AF.Tanh)
                if nb == 0:
                    nc.vector.tensor_add(rct[0:C, :, :], rct[0:C, :, :],
                                         cc_sb[:, u, :, :])

                # h update
                dd = sp.tile([128, BS, D], F32, tag="rcv")
                nc.vector.tensor_sub(dd[:], rct[:], h_sb[:, nb, :, :])
                d2 = sp.tile([128, BS, D], F32)
                nc.vector.tensor_mul(
                    d2[:], dd[:],
                    dec_sb[:, nb, :].unsqueeze(2).to_broadcast(
                        (128, BS, D)))
                nc.vector.tensor_add(h_sb[:, nb, :, :], h_sb[:, nb, :, :],
                                     d2[:])
                pmt = sp.tile([128, BS, D], F32)
                nc.vector.tensor_mul(pmt[:], h_sb[:, nb, :, :],
                                     effp_sb[:, nb, :, :])
                nc.scalar.activation(pm_sb[:, nb, :, :], pmt[:], AF.Tanh)
                if nb == 0:
                    outu = sp.tile([C, BS, D], F32, tag="outu")
                    nc.scalar.activation(outu[:], pmt[0:C, :, :], AF.Tanh)
                    nc.sync.dma_start(out=out_t.ap()[:, u], in_=outu[:])
                if u + 1 < U:
                    nc.sync.dma_start(out=pm_slices[nb].ap(),
                                      in_=pm_sb[:, nb, :, :])
                    nc.gpsimd.collective_compute(
                        "AllGather", OP.bypass,
                        replica_groups=[list(range(NCORES))],
                        ins=[pm_slices[nb].ap().opt()],
                        outs=[pm_full.ap()[nb].opt()])

    nc.compile()
    return nc


def prep_phase2_ns_inputs(c, eff_key, eff_prim, eff_decay, h, prev_messages,
                          cc_signals, conn, dendrite_branch_w,
                          dendrite_group_w, update_ts):
    """Per-core (neuron-slice c) input map for N-sharded phase 2."""
    NBL2 = 4
    S = slice(c * NS, (c + 1) * NS)

    def nb_layout(x):  # [NS, ...] -> [128, NBL2, ...]
        return np.ascontiguousarray(
            x.reshape((NBL2, 128) + x.shape[1:]).swapaxes(0, 1))

    def nb_layout_b(x):  # [BS, NS, ...] -> [128, NBL2, BS, ...]
        x = np.moveaxis(x, 0, 1)  # [NS, BS, ...]
        return nb_layout(x)

    w = dendrite_branch_w.reshape(N, K, D)[S]
    g = dendrite_group_w.reshape(N, BPG, D)[S]
    cs = conn[S]  # [NS, K]
    nmap = ((conn % 512) // 128) * (NCORES * 128) + \
        (conn // 512) * 128 + (conn % 128)  # row in pm_full order
    csm = nmap[S]
    idx = np.ascontiguousarray(
        csm.reshape(NBL2, 128, NB, BSZ).transpose(0, 2, 3, 1)
        .reshape(NBL2 * NB, BSZ, 128)).astype(np.int16)
    # dma_gather writes row i -> out partition i%128, chunk i//128; want
    # mg[p, k] = pm[conn[nb*128+p, j*8+k]] -> i = k*128 + p. Index i is
    # read from wrapped layout [16, num_idxs/16] replicated to 128 parts.
    idx_w = np.empty((128, NBL2 * NB, BSZ * 128 // 16), np.int16)
    for q in range(NBL2 * NB):
        flat = idx[q].reshape(-1)  # k-major, p inner
        wrap = flat.reshape(-1, 16).T  # [16, n/16]
        idx_w[:, q, :] = np.tile(wrap, (8, 1))
    cc = np.zeros((C, len(update_ts), BS, D), bf16)
    if c == 0:
        cc = np.ascontiguousarray(
            cc_signals[:, update_ts].transpose(2, 1, 0, 3)).astype(bf16)
    return {
        "pm_init": np.ascontiguousarray(
            np.moveaxis(prev_messages, 0, 1).reshape(NCORES, NBL2, 128,
                                                     BS, D)
            .transpose(1, 0, 2, 3, 4).reshape(N, BS, D)).astype(bf16),
        "w_nb": nb_layout(w).astype(bf16),
        "key_nb": nb_layout_b(eff_key[:, S]).astype(bf16),
        "effp_nb": nb_layout_b(eff_prim[:, S]).astype(np.float32),
        "dec1m_nb": nb_layout_b(1.0 - eff_decay[:, S]).astype(np.float32),
        "h0_nb": nb_layout_b(h[:, S]).astype(np.float32),
        "g_nb": nb_layout(g).astype(bf16),
        "cc_u": cc,
        "idx": np.ascontiguousarray(idx_w),
    }


# --------------------------------------------------------------------------
# Phase 2 v3: 4-way neuron x 2-way batch shard.
#
# Core c = bg*4 + ng owns neurons [ng*1024, (ng+1)*1024) and batches
# [bg*4, bg*4+4). Per update:
#   - dma_gather of neighbor pm rows (512B elems, full DMA bandwidth) into
#     partition layout p=(nsub16, s8), free=(nblk8, j2, b4, d64)
#   - sim/dot on DVE in bf16 2x mode (mult + halving tree)
#   - sigma folded into PE matmuls: stationary = wm rows [128,(d)64],
#     moving = sigma * block-diag mask [128,16]; psum out [d64, nsub16] at
#     partition offset 64*j2 via tile_position. The PE thus performs both
#     the sigma gating multiply and the branch-sum.
#   - ACT tanh evacuates psum into (j2,d)-partition layout; group combine +
#     h update run d-major; XBAR dma transpose re-partitions pm rows for
#     the next update's gather source.
#   - pm exchange between the 4 cores of a batch group via per-2-block
#     AllGathers (overlapped with compute).
# Last update computes only neurons 0..63 (the only ones observable).
# --------------------------------------------------------------------------
NG4, BG2 = 4, 2       # 4-way neuron shard x 2-way batch shard
NL3 = N // NG4        # neurons per core (1024)
BC3 = BS // BG2       # batches per core (4)
NB3 = NL3 // 128      # 128-neuron blocks per core (8)


def build_phase2_v3(U):
    nc = bacc.Bacc("TRN2", target_bir_lowering=False, debug=False,
                   num_devices=NCORES)
    EL = BC3 * D  # gather elem (256 elems = 512B bf16)

    pm_init = nc.dram_tensor("pm_init", [N, EL], BF16, kind="ExternalInput")
    idx_in = nc.dram_tensor("idx", [128, 2 * NB3, 128], I16,
                            kind="ExternalInput")
    key_in = nc.dram_tensor("key_r", [128, NB3, 8, BC3, D], BF16,
                            kind="ExternalInput")
    w_in = nc.dram_tensor("w_r", [128, NB3, 2, 8, 2, D], BF16,
                          kind="ExternalInput")
    mask_in = nc.dram_tensor("mask_r", [128, 64, 16], BF16,
                             kind="ExternalInput")
    g_in = nc.dram_tensor("g_r", [64, NB3, 2, 8, 2, 16], BF16,
                          kind="ExternalInput")
    h_in = nc.dram_tensor("h_d", [64, NB3, 8, 16, BC3], F32,
                          kind="ExternalInput")
    dec_in = nc.dram_tensor("dec1m_d", [64, NB3, 8, 16, BC3], BF16,
                            kind="ExternalInput")
    effp_in = nc.dram_tensor("effp_d", [64, NB3, 8, 16, BC3], F32,
                             kind="ExternalInput")
    cc_in = nc.dram_tensor("cc_d", [64, U, 4, 16, BC3], F16,
                           kind="ExternalInput")
    out_t = nc.dram_tensor("out_pm", [U, 64, 4, 16, BC3], F32,
                           kind="ExternalOutput")
    # exchange buffers: rows r = ch*1024 + ng'*256 + within.
    # Ping-pong pair so update u+1's exchange (writes) never carries a WAR
    # hazard against update u+1's own gathers (reads of the u buffer).
    pm_dram = [nc.dram_tensor(f"pm_dram{p}", [4, NG4, 256, EL], BF16)
               for p in range(2)]
    coll_in = [[nc.dram_tensor(f"coll_in{p}_{ch}", [256, EL], BF16)
                for ch in range(4)] for p in range(2)]
    groups = [[0, 1, 2, 3], [4, 5, 6, 7]]

    with tile.TileContext(nc) as tc, ExitStack() as ctx:
        res = ctx.enter_context(tc.tile_pool(name="res", bufs=1))
        gp = ctx.enter_context(tc.tile_pool(name="gath", bufs=3))
        wp = ctx.enter_context(tc.tile_pool(name="work", bufs=2))
        npool = ctx.enter_context(tc.tile_pool(name="nb", bufs=2))
        hp = ctx.enter_context(tc.tile_pool(name="hch", bufs=1))
        pqp = ctx.enter_context(tc.tile_pool(name="pq", bufs=2,
                                             space="PSUM"))
        xp = ctx.enter_context(tc.tile_pool(name="xbar", bufs=2))

        nc.gpsimd.load_library(library_config.mlp)
        idx_sb = res.tile([128, 2 * NB3, 128], I16)
        nc.sync.dma_start(out=idx_sb[:], in_=idx_in.ap())
        key_sb = res.tile([128, NB3, 8, BC3, D], BF16)
        nc.sync.dma_start(out=key_sb[:], in_=key_in.ap())
        w_sb = res.tile([128, NB3, 2, 8, 2, D], BF16)
        nc.sync.dma_start(out=w_sb[:], in_=w_in.ap())
        mask_sb = res.tile([128, 64, 16], BF16)
        nc.sync.dma_start(out=mask_sb[:], in_=mask_in.ap())
        g_sb = res.tile([64, NB3, 2, 8, 2, 16], BF16)
        nc.sync.dma_start(out=g_sb[:], in_=g_in.ap())
        h_sb = res.tile([64, NB3, 8, 16, BC3], F32)
        nc.sync.dma_start(out=h_sb[:], in_=h_in.ap())
        dec_sb = res.tile([64, NB3, 8, 16, BC3], BF16)
        nc.sync.dma_start(out=dec_sb[:], in_=dec_in.ap())
        effp_sb = res.tile([64, NB3, 8, 16, BC3], F32)
        nc.sync.dma_start(out=effp_sb[:], in_=effp_in.ap())
        cc_sb = res.tile([64, U, 4, 16, BC3], F16)
        nc.sync.dma_start(out=cc_sb[:], in_=cc_in.ap())

        pm_chs = {}

        def emit_gather(nb, jj, NK, src):
            NIDX = NK * 2 * 128
            mg = gp.tile([128, NK, 2, BC3, D], BF16, tag="mg")
            nc.gpsimd.dma_gather(
                out_ap=mg[:].rearrange("p n j b d -> p (n j) (b d)"),
                in_ap=src, idxs_ap=idx_sb[:, nb * 2 + jj, :],
                num_idxs=NIDX, num_idxs_reg=NIDX, elem_size=EL,
                single_packet=False)
            return mg

        def emit_qsim(u, nb, jj, NK, mg):
            # sim = <msg, key> over d (bf16 tree)
            tmp = wp.tile([128, NK, 2, BC3, D], BF16, tag="tmp")
            keyb = key_sb[:, nb, 0:NK].unsqueeze(2).to_broadcast(
                (128, NK, 2, BC3, D))
            nc.vector.tensor_mul(tmp[:], mg[:], keyb)
            r1 = wp.tile([128, NK, 2, BC3, 32], BF16, tag="r1")
            nc.vector.tensor_add(r1[:], tmp[:, :, :, :, 0:32],
                                 tmp[:, :, :, :, 32:64])
            r2 = wp.tile([128, NK, 2, BC3, 16], BF16, tag="r2")
            nc.vector.tensor_add(r2[:], r1[:, :, :, :, 0:16],
                                 r1[:, :, :, :, 16:32])
            r3 = wp.tile([128, NK, 2, BC3, 8], BF16, tag="r3")
            nc.vector.tensor_add(r3[:], r2[:, :, :, :, 0:8],
                                 r2[:, :, :, :, 8:16])
            r4 = wp.tile([128, NK, 2, BC3, 4], BF16, tag="r4")
            nc.vector.tensor_add(r4[:], r3[:, :, :, :, 0:4],
                                 r3[:, :, :, :, 4:8])
            r5 = wp.tile([128, NK, 2, BC3, 2], BF16, tag="r5")
            nc.vector.tensor_add(r5[:], r4[:, :, :, :, 0:2],
                                 r4[:, :, :, :, 2:4])
            sim = wp.tile([128, NK, 2, BC3, 1], F32, tag="sim")
            nc.vector.tensor_add(sim[:], r5[:, :, :, :, 0:1],
                                 r5[:, :, :, :, 1:2])
            # sigma expanded to 16 lanes on ACT (emitted here so it is
            # long done before the mm-stage's DVE mask-mult needs it)
            sg = wp.tile([128, NK, 2, BC3, 16], BF16, tag="sg")
            nc.scalar.activation(
                sg[:], sim[:].to_broadcast((128, NK, 2, BC3, 16)),
                AF.Sigmoid)
            # wm = msg * w (branch weights), in-place into mg
            wm = mg
            wb = w_sb[:, nb, jj, 0:NK].unsqueeze(3).to_broadcast(
                (128, NK, 2, BC3, D))
            nc.vector.tensor_mul(wm[:], mg[:], wb)
            return wm, sg

        def emit_qmm(u, nb, jj, NK, wm, sg, brt):
            # mask sigma to the block-diagonal PE stationary (in-place)
            lhsT = sg
            mv = mask_sb[:, 0:NK * 2 * BC3, :].rearrange(
                "p (n j b) i -> p n j b i", n=NK, j=2)
            nc.vector.tensor_mul(lhsT[:], mv, sg[:])
            # psum laid out b-outer so the later group-weight broadcast
            # stays within the 3-free-dim ISA limit
            pq = pqp.tile([64, BC3, NK, 2, 16], F32)
            for nblk in range(NK):
                for j2 in range(2):
                    for b in range(BC3):
                        nc.tensor.matmul(
                            pq[:, b, nblk, j2, :],
                            wm[:, nblk, j2, b, :],
                            lhsT[:, nblk, j2, b, :],
                            start=True, stop=True)
            nc.scalar.activation(brt[:, jj], pq[:], AF.Tanh)

        def emit_A(u, nb, NK, brt, eng=None):
            eng = eng or nc.gpsimd
            # group combine (weights g), then j-sums; returns rct
            gb = brt  # in-place: brt dead after the group multiply
            gball = g_sb[:, nb, :, 0:NK].unsqueeze(2).to_broadcast(
                (64, 2, BC3, NK, 2, 16))
            nc.vector.tensor_mul(gb[:], brt[:], gball)
            gs = gb  # in-place jj-sum into gb[:, 0]
            nc.vector.tensor_add(gs[:, 0], gb[:, 0], gb[:, 1])
            rcv = hp.tile([64, NK, 16, BC3], F32, tag="rcv")
            eng.tensor_add(
                rcv[:].rearrange("p n s b -> p b n s"),
                gs[:, 0, :, :, 0], gs[:, 0, :, :, 1])
            rct = npool.tile([64, NK, 16, BC3], F16, tag="rct")
            nc.scalar.activation(rct[:], rcv[:], AF.Tanh)
            if nb == 0:  # cc is added AFTER the dendritic tanh
                eng.tensor_add(rct[:, 0:4], rct[:, 0:4], cc_sb[:, u])
            return rct

        def emit_B(u, nb, NK, last, rct, eng=None):
            eng = eng or nc.gpsimd
            # h' = h + (1-e)*(rct - h);  pm = tanh(h' * effp)
            dd = hp.tile([64, NK, 16, BC3], F32, tag="dd")
            eng.tensor_sub(dd[:], rct[:], h_sb[:, nb, 0:NK])
            d2 = hp.tile([64, NK, 16, BC3], F32, tag="d2")
            eng.tensor_mul(d2[:], dd[:], dec_sb[:, nb, 0:NK])
            eng.tensor_add(h_sb[:, nb, 0:NK], h_sb[:, nb, 0:NK],
                           d2[:])
            pmt = hp.tile([64, NK, 16, BC3], F32, tag="pmt")
            eng.tensor_mul(pmt[:], h_sb[:, nb, 0:NK],
                           effp_sb[:, nb, 0:NK])
            if nb == 0:
                outu = hp.tile([64, 4, 16, BC3], F32, tag="outu")
                nc.scalar.activation(outu[:], pmt[:, 0:4], AF.Tanh)
                nc.sync.dma_start(out=out_t.ap()[u], in_=outu[:])
            if not last:
                if nb % 2 == 0:
                    pm_ch = xp.tile([64, 2, NB3, 16, BC3], BF16,
                                    tag="pmch")
                    pm_chs[nb // 2] = pm_ch
                pm_ch = pm_chs[nb // 2]
                nc.scalar.activation(pm_ch[:, nb % 2], pmt[:], AF.Tanh)

        def emit_C(u, ch):
            # XBAR re-partition + DRAM write + AllGather for chunk ch
            par = u % 2
            pm_ch = pm_chs.pop(ch)
            pmT = xp.tile([128, 8, 64], BF16, tag="pmT")
            nc.sync.dma_start_transpose(
                out=pmT[:],
                in_=pm_ch[:].rearrange("p a n s b -> p (a n s b)"))
            # pmT[p, c, d] = pm col (c*128+p); col=(nl2, b) with
            # nl2 = c*32 + p//4, b = p%4 -> 4 partition-strided
            # writes into the collective input rows.
            for r in range(BC3):
                src_ap = pmT[:].rearrange(
                    "(a r) c d -> a r c d", r=BC3)[:, r]
                dst = coll_in[par][ch].ap()[:, r * D:(r + 1) * D] \
                    .rearrange("(c a) d -> a c d", c=8)
                nc.sync.dma_start(out=dst, in_=src_ap)
            nc.gpsimd.collective_compute(
                "AllGather", OP.bypass, replica_groups=groups,
                ins=[coll_in[par][ch].ap().opt()],
                outs=[pm_dram[par].ap()[ch].opt()])

        for u in range(U):
            last = (u == U - 1)
            src = (pm_init.ap() if u == 0 else
                   pm_dram[(u - 1) % 2].ap()
                   .rearrange("c g w e -> (c g w) e"))
            NK = 4 if last else 8   # nblk count (truncated last update)
            if last:
                brt = npool.tile([64, 2, BC3, NK, 2, 16], BF16, tag="brt")
                mg0 = emit_gather(0, 0, NK, src)
                mg1 = emit_gather(0, 1, NK, src)
                wm0, sg0 = emit_qsim(u, 0, 0, NK, mg0)
                emit_qmm(u, 0, 0, NK, wm0, sg0, brt)
                wm1, sg1 = emit_qsim(u, 0, 1, NK, mg1)
                emit_qmm(u, 0, 1, NK, wm1, sg1, brt)
                rct = emit_A(u, 0, NK, brt)
                emit_B(u, 0, NK, last, rct)
                continue
            # software pipeline with lagged stages so every emitted
            # instruction's inputs are already (nearly) ready -- the
            # in-order engine queues then never stall behind a stage.
            brts, rcts = {}, {}
            for nb in range(NB3):
                mg0 = emit_gather(nb, 0, NK, src)
                mg1 = emit_gather(nb, 1, NK, src)
                if nb >= 1:
                    rcts[nb - 1] = emit_A(u, nb - 1, NK, brts.pop(nb - 1))
                brt = npool.tile([64, 2, BC3, NK, 2, 16], BF16, tag="brt")
                wm0, sg0 = emit_qsim(u, nb, 0, NK, mg0)
                wm1, sg1 = emit_qsim(u, nb, 1, NK, mg1)
                emit_qmm(u, nb, 0, NK, wm0, sg0, brt)
                emit_qmm(u, nb, 1, NK, wm1, sg1, brt)
                brts[nb] = brt
                if nb >= 2:
                    emit_B(u, nb - 2, NK, last, rcts.pop(nb - 2))
                    if nb % 2 == 1:
                        emit_C(u, (nb - 2) // 2)
            rcts[NB3 - 1] = emit_A(u, NB3 - 1, NK, brts.pop(NB3 - 1),
                                   eng=nc.vector)
            emit_B(u, NB3 - 2, NK, last, rcts.pop(NB3 - 2), eng=nc.vector)
            emit_B(u, NB3 - 1, NK, last, rcts.pop(NB3 - 1), eng=nc.vector)
            emit_C(u, 3)

    nc.compile()
    return nc


_ROWMAP = None


def _rowmap():
    """Global neuron id -> row in pm_dram/pm_init layout."""
    global _ROWMAP
    if _ROWMAP is None:
        g = np.arange(N)
        ngp, nl = g // NL3, g % NL3
        ch = nl // 256
        _ROWMAP = ch * 1024 + ngp * 256 + (nl - ch * 256)
    return _ROWMAP


def prep_phase2_v3_inputs(c, eff_key, eff_prim, eff_decay, h, prev_messages,
                          cc_signals, conn, dendrite_branch_w,
                          dendrite_group_w, update_ts):
    ng, bg = c % NG4, c // NG4
    S = slice(ng * NL3, (ng + 1) * NL3)
    Bs = slice(bg * BC3, (bg + 1) * BC3)
    U = len(update_ts)
    f32, b16 = np.float32, bf16

    # gather idx: value = rowmap[conn[n, slot]]; order per q=(nb,jj):
    # i = (nblk*2+j2)*128 + (nsub*8+s)
    rows = _rowmap()[conn[S]].astype(np.int64)  # [1024, 32]
    R = rows.reshape(NB3, 8, 16, 4, 8)  # [nb, nblk, nsub, j, s]
    idx_w = np.empty((128, 2 * NB3, 128), np.int16)
    for nb in range(NB3):
        for jj in range(2):
            sub = R[nb, :, :, 2 * jj:2 * jj + 2, :]  # [nblk, nsub, j2, s]
            flat = np.ascontiguousarray(
                sub.transpose(0, 2, 1, 3)).reshape(-1)  # (nblk,j2),(nsub,s)
            wrap = flat.reshape(-1, 16).T  # [16, 128]
            idx_w[:, nb * 2 + jj, :] = np.tile(wrap, (8, 1))

    pm_init = np.empty((N, BC3 * D), f32)
    pm_init[_rowmap()] = np.ascontiguousarray(
        prev_messages[Bs].transpose(1, 0, 2)).reshape(N, BC3 * D)

    ek = eff_key[Bs][:, S]  # [4, 1024, 64]
    E = ek.transpose(1, 0, 2).reshape(NB3, 8, 16, BC3, D)
    key_r = np.broadcast_to(
        E.transpose(2, 0, 1, 3, 4)[:, None],
        (16, 8, NB3, 8, BC3, D)).reshape(128, NB3, 8, BC3, D)

    w = dendrite_branch_w[S]  # [1024, 4, 8, 64]
    W = w.reshape(NB3, 8, 16, 4, 8, D)
    w_r = np.ascontiguousarray(
        W.transpose(2, 4, 0, 3, 1, 5).reshape(128, NB3, 4, 8, D)
        .reshape(128, NB3, 2, 2, 8, D).transpose(0, 1, 2, 4, 3, 5))

    ii = np.arange(16)
    mask_r = np.broadcast_to(
        (ii[:, None, None, None] == ii[None, None, None, :]),
        (16, 8, 64, 16)).reshape(128, 64, 16).astype(f32)

    gw = dendrite_group_w[S][:, 0]  # [1024, 4, 64]
    G = gw.reshape(NB3, 8, 16, 4, D)
    # [d, nb, jj, nblk, j2, nsub]
    g_r = np.ascontiguousarray(
        G.transpose(4, 0, 3, 1, 2).reshape(D, NB3, 2, 2, 8, 16)
        .transpose(0, 1, 2, 4, 3, 5))

    def dmaj(x):  # [4b, 1024, 64] -> [64, nb, nblk, nsub, b]
        return np.ascontiguousarray(
            x.reshape(BC3, NB3, 8, 16, D).transpose(4, 1, 2, 3, 0))

    h_d = dmaj(h[Bs][:, S])
    effp_d = dmaj(eff_prim[Bs][:, S])
    dec1 = (1.0 - eff_decay[Bs][:, S]).reshape(BC3, NB3, 8, 16)
    dec_d = np.broadcast_to(dec1.transpose(1, 2, 3, 0)[None],
                            (D, NB3, 8, 16, BC3))

    if ng == 0:
        cc = cc_signals[Bs][:, update_ts]  # [4b, U, 64, 64]
        cc_d = np.ascontiguousarray(
            cc.reshape(BC3, U, 4, 16, D).transpose(4, 1, 2, 3, 0))
    else:
        cc_d = np.zeros((D, U, 4, 16, BC3), f32)

    return {
        "pm_init": pm_init.astype(b16),
        "idx": np.ascontiguousarray(idx_w),
        "key_r": np.ascontiguousarray(key_r).astype(b16),
        "w_r": w_r.astype(b16),
        "mask_r": mask_r.astype(b16),
        "g_r": g_r.astype(b16),
        "h_d": h_d.astype(f32),
        "dec1m_d": np.ascontiguousarray(dec_d).astype(b16),
        "effp_d": effp_d.astype(f32),
        "cc_d": cc_d.astype(np.float16),
    }


# --------------------------------------------------------------------------
# Phase 1: N-sharded modulator MLP
# --------------------------------------------------------------------------
def build_phase1(NSH=NS):
    """NSH neurons per core, all BS batches."""
    nc = bacc.Bacc("TRN2", target_bir_lowering=False, debug=False,
                   num_devices=NCORES)
    NP = NSH // 2  # pairs

    # weights host-prearranged partition-major so loads are few big DMAs
    fc1a = nc.dram_tensor("fc1a", [128, NSH, 2, H], BF16,
                          kind="ExternalInput")
    fc1c = nc.dram_tensor("fc1c", [64, NSH, H], BF16, kind="ExternalInput")
    fc1b = nc.dram_tensor("fc1b", [128, NP], F32, kind="ExternalInput")
    fc2p = nc.dram_tensor("fc2p", [128, NP, 6], BF16, kind="ExternalInput")
    fc2b = nc.dram_tensor("fc2b", [BS, NP, 6], F32, kind="ExternalInput")
    modc0 = nc.dram_tensor("modc0", [128, NSH, BS], BF16,
                           kind="ExternalInput")
    modc1 = nc.dram_tensor("modc1", [128, NSH, BS], BF16,
                           kind="ExternalInput")
    modc2 = nc.dram_tensor("modc2", [64, NSH, BS], BF16,
                           kind="ExternalInput")
    NBL = NSH // 128
    tp_n = nc.dram_tensor("tp_n", [128, NBL, BS, D], F32,
                          kind="ExternalInput")
    tk_n = nc.dram_tensor("tk_n", [128, NBL, BS, D], F32,
                          kind="ExternalInput")
    prim_n = nc.dram_tensor("prim_n", [128, NBL, D], F32,
                            kind="ExternalInput")
    keyp_n = nc.dram_tensor("keyp_n", [128, NBL, D], F32,
                            kind="ExternalInput")
    dlog_n = nc.dram_tensor("dlog_n", [128, NBL], F32, kind="ExternalInput")
    mllog = nc.dram_tensor("mllog", [1, 1], F32, kind="ExternalInput")

    effp_o = nc.dram_tensor("effp_o", [128, NBL, BS, D], F32,
                            kind="ExternalOutput")
    effk_o = nc.dram_tensor("effk_o", [128, NBL, BS, D], F32,
                            kind="ExternalOutput")
    dec_o = nc.dram_tensor("dec_o", [128, NBL, BS], F32,
                           kind="ExternalOutput")

    with tile.TileContext(nc) as tc, ExitStack() as ctx:
        res = ctx.enter_context(tc.tile_pool(name="res", bufs=1))
        dram = ctx.enter_context(tc.tile_pool(name="dram", bufs=1,
                                              space="DRAM"))
        wpool = ctx.enter_context(tc.tile_pool(name="wts", bufs=2))
        ps = ctx.enter_context(tc.tile_pool(name="ps", bufs=2, space="PSUM"))
        ps2 = ctx.enter_context(tc.tile_pool(name="ps2", bufs=2,
                                             space="PSUM"))
        sp = ctx.enter_context(tc.tile_pool(name="small", bufs=2))

        m0 = res.tile([128, NSH, BS], BF16)
        nc.sync.dma_start(out=m0[:], in_=modc0.ap())
        m1 = res.tile([128, NSH, BS], BF16)
        nc.sync.dma_start(out=m1[:], in_=modc1.ap())
        m2 = res.tile([64, NSH, BS], BF16)
        nc.sync.dma_start(out=m2[:], in_=modc2.ap())
        fb1 = res.tile([128, NP], F32)
        nc.sync.dma_start(out=fb1[:], in_=fc1b.ap())
        fb2 = res.tile([BS, NP, 6], F32)
        nc.sync.dma_start(out=fb2[:], in_=fc2b.ap())
        x_sb = res.tile([128, NP, BS], BF16)
        o_sb = res.tile([BS, NP, 6], F32)
        ml_sb = sp.tile([1, 1], F32)
        nc.sync.dma_start(out=ml_sb[:], in_=mllog.ap())
        ones_r = sp.tile([1, 128], F32)
        nc.vector.memset(ones_r[:], 1.0)
        lr_ps = ps2.tile([128, 1], F32, space="PSUM")
        lrs = sp.tile([1, 1], F32)
        nc.scalar.activation(lrs[:], ml_sb[:], AF.Sigmoid)
        nc.tensor.matmul(lr_ps[:], ones_r[:], lrs[:], start=True, stop=True)
        lr128 = res.tile([128, 1], F32)
        nc.vector.tensor_copy(lr128[:], lr_ps[:])

        # --- fc1: per neuron, 3 contraction chunks -> psum [(h,par), ...] ---
        SEC = 64  # neurons per weight section
        GRP = 32   # pairs per psum tile (= SEC neurons)
        fc2w_sb = res.tile([128, NP, 6], BF16)
        nc.sync.dma_start(out=fc2w_sb[:], in_=fc2p.ap())
        for g in range(NSH // SEC):
            wa = wpool.tile([128, SEC, 2, H], BF16, tag="wa")
            nc.sync.dma_start(out=wa[:],
                              in_=fc1a.ap()[:, g * SEC:(g + 1) * SEC])
            wc = wpool.tile([64, SEC, H], BF16, tag="wc")
            nc.sync.dma_start(out=wc[:],
                              in_=fc1c.ap()[:, g * SEC:(g + 1) * SEC])
            pst = ps.tile([128, GRP * 8], F32, space="PSUM")
            for jj in range(GRP):
                for par in range(2):
                    nl = 2 * jj + par
                    n = g * SEC + nl
                    o = pst[64 * par:64 * par + 64, 8 * jj:8 * jj + 8]
                    tpos = (0, 64) if par else None
                    nc.tensor.matmul(o, wa[:, nl, 0, :], m0[:, n, :],
                                     start=True, stop=False,
                                     tile_position=tpos)
                    nc.tensor.matmul(o, wa[:, nl, 1, :], m1[:, n, :],
                                     start=False, stop=False,
                                     tile_position=tpos)
                    nc.tensor.matmul(o, wc[:, nl, :], m2[:, n, :],
                                     start=False, stop=True,
                                     tile_position=tpos)
            xb = sp.tile([128, GRP, BS], F32, tag="xb")
            nc.vector.tensor_add(
                xb[:], pst[:].rearrange("p (j b) -> p j b", b=BS),
                fb1[:, g * GRP:(g + 1) * GRP].unsqueeze(2).to_broadcast(
                    (128, GRP, BS)))
            nc.scalar.activation(x_sb[:, g * GRP:(g + 1) * GRP, :], xb[:],
                                 AF.Tanh)

        # --- fc2: per pair, block-diagonal rhs ---
        G2 = 64
        for g in range(NP // G2):
            pst = ps2.tile([BS, G2 * 6], F32, space="PSUM")
            for jj in range(G2):
                pair = g * G2 + jj
                nc.tensor.matmul(pst[:, 6 * jj:6 * jj + 6],
                                 x_sb[:, pair, :], fc2w_sb[:, pair, :],
                                 start=True, stop=True)
            nc.vector.tensor_add(
                o_sb[:, g * G2:(g + 1) * G2, :],
                pst[:].rearrange("p (j o) -> p j o", o=6),
                fb2[:, g * G2:(g + 1) * G2, :])

        # --- reshuffle gates to n-major via DRAM round trip ---
        o_dram = dram.tile([BS, NP, 6], F32)
        nc.sync.dma_start(out=o_dram[:, :, :], in_=o_sb[:])
        gn = res.tile([128, NBL, BS, 3], F32)
        # o_dram[b, pair, par*3+o]; pair = nb*64 + p//2, par = p%2
        # (p2 par) merges to partition stride 3; one DMA per batch keeps
        # the AP within the 3-axis DMA limit.
        for b in range(BS):
            nc.sync.dma_start(
                out=gn[:, :, b, :],
                in_=o_dram[b, :, :].rearrange(
                    "(nb p2) (par o) -> (p2 par) nb o", nb=NBL, par=2))

        # --- trace direction normalization ---
        tps = res.tile([128, NBL, BS, D], F32)
        nc.sync.dma_start(out=tps[:], in_=tp_n.ap())
        tks = res.tile([128, NBL, BS, D], F32)
        nc.sync.dma_start(out=tks[:], in_=tk_n.ap())
        pr_s = res.tile([128, NBL, D], F32)
        nc.sync.dma_start(out=pr_s[:], in_=prim_n.ap())
        kp_s = res.tile([128, NBL, D], F32)
        nc.sync.dma_start(out=kp_s[:], in_=keyp_n.ap())
        dl_s = res.tile([128, NBL], F32)
        nc.sync.dma_start(out=dl_s[:], in_=dlog_n.ap())

        def assemble(trace, base_ap, gate_col, out_ap):
            sq = sp.tile([128, NBL, BS, D], F32, tag="sq")
            nc.vector.tensor_mul(sq[:], trace[:], trace[:])
            ss = sp.tile([128, NBL, BS], F32, tag="ss")
            nc.vector.tensor_reduce(ss[:], sq[:], axis=mybir.AxisListType.X,
                                    op=OP.add)
            nrm = sp.tile([128, NBL, BS], F32, tag="nrm")
            nc.scalar.activation(nrm[:], ss[:], AF.Sqrt)
            nc.vector.tensor_scalar(nrm[:], nrm[:], 1e-8, None, OP.max)
            rn = sp.tile([128, NBL, BS], F32, tag="rn")
            nc.vector.reciprocal(rn[:], nrm[:])
            # s = lr * tanh(gate)
            gt = sp.tile([128, NBL, BS], F32, tag="gt")
            nc.scalar.activation(gt[:], gn[:, :, :, gate_col], AF.Tanh)
            nc.vector.tensor_scalar(gt[:], gt[:], lr128[:, 0:1], None,
                                    OP.mult)
            nc.vector.tensor_mul(gt[:], gt[:], rn[:])
            eo = sp.tile([128, NBL, BS, D], F32, tag="eo")
            nc.vector.tensor_mul(
                eo[:], trace[:],
                gt[:].unsqueeze(3).to_broadcast((128, NBL, BS, D)))
            nc.vector.tensor_add(
                eo[:], eo[:],
                base_ap.unsqueeze(2).to_broadcast((128, NBL, BS, D)))
            nc.sync.dma_start(out=out_ap, in_=eo[:])

        assemble(tps, pr_s[:], 0, effp_o.ap())
        assemble(tks, kp_s[:], 1, effk_o.ap())

        dd = sp.tile([128, NBL, BS], F32)
        nc.vector.tensor_add(
            dd[:], gn[:, :, :, 2],
            dl_s[:].unsqueeze(2).to_broadcast((128, NBL, BS)))
        de = sp.tile([128, NBL, BS], F32)
        nc.scalar.activation(de[:], dd[:], AF.Sigmoid)
        nc.sync.dma_start(out=dec_o.ap(), in_=de[:])

    nc.compile()
    return nc


def prep_phase1_inputs(c, h, trace_prim, trace_key, primitives, key_p,
                       decay_logit, fc1_w, fc1_b, fc2_w, fc2_b, mod_lr_logit,
                       NSH=NS):
    S = slice(c * NSH, (c + 1) * NSH)
    NP = NSH // 2
    NBL = NSH // 128
    f1 = fc1_w[S]  # [NSH, 320, H]
    fc1a = np.ascontiguousarray(
        f1[:, 0:256, :].reshape(NSH, 2, 128, H)
        .transpose(2, 0, 1, 3)).astype(bf16)  # [128, NSH, 2, H]
    fc1c = np.ascontiguousarray(
        f1[:, 256:320, :].transpose(1, 0, 2)).astype(bf16)  # [64, NSH, H]
    # fc1b arranged [128=(h,parity), pair]
    b1 = fc1_b[S].reshape(NP, 2, H)  # [pair, par, h]
    fc1b_a = np.ascontiguousarray(
        b1.transpose(1, 2, 0).reshape(128, NP)).astype(np.float32)
    # fc2 block-diag pairs: [128=(par,h), pair, 6]
    f2 = fc2_w[S].reshape(NP, 2, H, 3)
    fc2p = np.zeros((NP, 128, 6), np.float32)
    fc2p[:, 0:64, 0:3] = f2[:, 0, :, :]
    fc2p[:, 64:128, 3:6] = f2[:, 1, :, :]
    fc2p = np.ascontiguousarray(fc2p.transpose(1, 0, 2)).astype(bf16)
    fc2b_a = np.broadcast_to(
        fc2_b[S].reshape(1, NP, 6), (BS, NP, 6))
    fc2b_a = np.ascontiguousarray(fc2b_a).astype(np.float32)

    def transp(x):  # [BS, NSH, D] -> [D, NSH, BS]
        return np.ascontiguousarray(x.transpose(2, 1, 0))

    hT = transp(h[:, S, :])
    tpT = transp(trace_prim[:, S, :])
    tkT = transp(trace_key[:, S, :])
    prT = np.broadcast_to(primitives[S].T[:, :, None], (D, NSH, BS))
    kpT = np.broadcast_to(key_p[S].T[:, :, None], (D, NSH, BS))
    modc0 = np.concatenate([hT, tpT], axis=0).astype(bf16)
    modc1 = np.concatenate([tkT, prT], axis=0).astype(bf16)
    modc2 = np.ascontiguousarray(kpT).astype(bf16)

    def nb_layout(x):  # [NSH, ...] -> [128, NBL, ...]
        return np.ascontiguousarray(
            x.reshape((NBL, 128) + x.shape[1:]).swapaxes(0, 1))

    def nb_layout_b(x):  # [BS, NSH, D] -> [128, NBL, BS, D]
        return np.ascontiguousarray(
            x.reshape(BS, NBL, 128, D).transpose(2, 1, 0, 3))

    return {
        "fc1a": fc1a, "fc1c": fc1c, "fc1b": fc1b_a, "fc2p": fc2p,
        "fc2b": fc2b_a, "modc0": modc0, "modc1": modc1, "modc2": modc2,
        "tp_n": nb_layout_b(trace_prim[:, S, :]).astype(np.float32),
        "tk_n": nb_layout_b(trace_key[:, S, :]).astype(np.float32),
        "prim_n": nb_layout(primitives[S]).astype(np.float32),
        "keyp_n": nb_layout(key_p[S]).astype(np.float32),
        "dlog_n": nb_layout(decay_logit[S]).astype(np.float32),
        "mllog": np.asarray(mod_lr_logit, np.float32).reshape(1, 1),
    }


# --------------------------------------------------------------------------
# Top level
# --------------------------------------------------------------------------
P2_MODE = "v3"  # "v3" | "ns" | "b"


def kernel(**inputs):
    inp = {k: np.asarray(v) for k, v in inputs.items()}
    stride = int(inp["stride"])
    update_ts = [t for t in range(T) if t % stride == 0]
    U = len(update_ts)

    if "p1" not in _prog_cache:
        _prog_cache["p1"] = build_phase1()
    p2_key = ("p2" + P2_MODE, U)
    if p2_key not in _prog_cache:
        builder = {"v3": build_phase2_v3, "ns": build_phase2_ns,
                   "b": build_phase2}[P2_MODE]
        _prog_cache[p2_key] = builder(U)
    nc1 = _prog_cache["p1"]
    nc2 = _prog_cache[p2_key]

    # ---- phase 1 ----
    in_maps1 = [
        prep_phase1_inputs(c, inp["h"], inp["trace_prim"], inp["trace_key"],
                           inp["primitives"], inp["key_p"],
                           inp["decay_logit"], inp["fc1_w"], inp["fc1_b"],
                           inp["fc2_w"], inp["fc2_b"], inp["mod_lr_logit"])
        for c in range(NCORES)
    ]
    res1 = run_bass_kernel_spmd(nc1, in_maps1, core_ids=list(range(NCORES)))

    # outputs [128, NBL, BS, D] per core; n = core*NS + nb*128 + p
    NBL = NS // 128
    effp = np.concatenate([res1.results[c]["effp_o"] for c in range(NCORES)],
                          axis=1)  # [128, 32, BS, D]
    effk = np.concatenate([res1.results[c]["effk_o"] for c in range(NCORES)],
                          axis=1)
    dec = np.concatenate([res1.results[c]["dec_o"] for c in range(NCORES)],
                         axis=1)  # [128, 32, BS]

    # to [BS, N, D] logical order for phase-2 prep
    eff_prim = np.ascontiguousarray(effp.transpose(2, 1, 0, 3)).reshape(
        BS, N, D)
    eff_key = np.ascontiguousarray(effk.transpose(2, 1, 0, 3)).reshape(
        BS, N, D)
    eff_decay = np.ascontiguousarray(dec.transpose(2, 1, 0)).reshape(BS, N)

    # ---- phase 2 ----
    conn = inp["conn_indices"].astype(np.int64)
    if P2_MODE == "v3":
        in_maps2 = [
            prep_phase2_v3_inputs(c, eff_key, eff_prim, eff_decay, inp["h"],
                                  inp["prev_messages"], inp["cc_signals"],
                                  conn, inp["dendrite_branch_w"],
                                  inp["dendrite_group_w"], update_ts)
            for c in range(NCORES)
        ]
        res2 = run_bass_kernel_spmd(nc2, in_maps2,
                                    core_ids=list(range(NCORES)))
        out = np.empty((BS, T, C, D), np.float32)
        uts = np.asarray(update_ts)
        for bg in range(BG2):
            op = res2.results[bg * NG4]["out_pm"]  # [U, 64d, 4, 16, 4b]
            pm_c = op.transpose(0, 4, 2, 3, 1).reshape(U, BC3, C, D)
            for t in range(T):
                u = int(np.searchsorted(uts, t, side="right") - 1)
                out[bg * BC3:(bg + 1) * BC3, t] = pm_c[u]
        return out
    if P2_MODE == "ns":
        in_maps2 = [
            prep_phase2_ns_inputs(c, eff_key, eff_prim, eff_decay, inp["h"],
                                  inp["prev_messages"], inp["cc_signals"],
                                  conn, inp["dendrite_branch_w"],
                                  inp["dendrite_group_w"], update_ts)
            for c in range(NCORES)
        ]
        res2 = run_bass_kernel_spmd(nc2, in_maps2,
                                    core_ids=list(range(NCORES)))
        out = np.empty((BS, T, C, D), np.float32)
        uts = np.asarray(update_ts)
        op = res2.results[0]["out_pm"]  # [C, U, BS, D]
        for t in range(T):
            u = int(np.searchsorted(uts, t, side="right") - 1)
            out[:, t] = op[:, u].transpose(1, 0, 2)
        return out

    w_kmaj, g_nb = prep_phase2_consts(inp["dendrite_branch_w"],
                                      inp["dendrite_group_w"])
    in_maps2 = [
        prep_phase2_inputs(b, eff_key, eff_prim, eff_decay, inp["h"],
                           inp["prev_messages"], inp["cc_signals"], conn,
                           w_kmaj, g_nb, update_ts)
        for b in range(BS)
    ]
    res2 = run_bass_kernel_spmd(nc2, in_maps2, core_ids=list(range(NCORES)))

    # assemble output [BS, T, C, D]
    out = np.empty((BS, T, C, D), np.float32)
    uts = np.asarray(update_ts)
    for b in range(BS):
        op = res2.results[b]["out_pm"]  # [C, U, D]
        for t in range(T):
            u = int(np.searchsorted(uts, t, side="right") - 1)
            out[b, t] = op[:, u, :]
    return out

